# revision 13
# baseline (speedup 1.0000x reference)
"""Trainium2 Bass kernel for nn_C2CFuser (C2C fuse branch + squeeze-excite + gumbel gate).

Math (per layer l, head h, branch X in {K, V}):
    z      = concat(X_R, X_S) @ (Wp@W1) + (bp@W1 + b1)          # merged first two linears
    hact   = gelu_exact(z)
    fused  = hact @ W2 + b2
    pooled = mean_{s,d} fused                                    # per (l, h)
    scale  = sigmoid(Wdw @ pooled + bdw)                         # mixes heads within a layer
    out    = X_R + gate_l * scale_h * fused

Sharding: 48 independent (layer, branch) units, 6 per core (cores 0-3: K branch,
cores 4-7: V branch).  All cores run one SPMD NEFF with different input bindings.

On-chip layout: activations are kept feature-on-partition ("transposed") so every
matmul streams the activation as the moving operand with the tiny weights
stationary (float32r -> 1 cycle/row at N=512).  Inputs are transposed on the PE
(128x128 blocks).  The residual is added in transposed space and the output is
written d-major; the host un-permutes at the end.

SBUF free-dim index j within a head encodes s = p*NT + n0 where j = n0*128 + p
(NT = S/128).  The load DMA access pattern produces it; the host undoes it.
"""

import sys

for _p in ("/opt/trn_rl_repo",):
    if _p not in sys.path:
        sys.path.insert(0, _p)

import numpy as np

import concourse.bass as bass
import concourse.bacc as bacc
import concourse.tile as tile
from concourse import mybir
from concourse.bass_utils import run_bass_kernel_spmd

F32 = mybir.dt.float32
F32R = mybir.dt.float32r
BF16 = mybir.dt.bfloat16
AF = mybir.ActivationFunctionType
OP = mybir.AluOpType

P = 128          # partitions / head dim
W = 512          # s-tile width (one PSUM bank of fp32)

# jax.random.uniform(jax.random.key(42), (24,), float32) -- fixed gumbel noise
_GUMBEL_U = np.array([
    0.59400654, 0.43801308, 0.6285691, 0.00791204, 0.27834702, 0.7976179,
    0.8521497, 0.9625306, 0.67656493, 0.11104441, 0.4959929, 0.7311437,
    0.18970704, 0.1544199, 0.03802836, 0.33559263, 0.92825687, 0.6123972,
    0.49262476, 0.733806, 0.18920851, 0.15386605, 0.037136197, 0.32930005,
], dtype=np.float32)
_EPS = np.float32(1e-10)
_TEMPERATURE = np.float32(0.5)


def build_module(BL: int, H: int, S: int, act_func=AF.Gelu):
    """Build the SPMD Bass module for BL branch-layers of H heads, seq len S."""
    NT = S // P            # 128-row chunks per head
    ST = S // W            # 512-wide s-tiles per head
    NC = H * ST            # pooled-partial columns per branch-layer

    nc = bacc.Bacc("TRN2", target_bir_lowering=False, debug=False)

    XR = nc.dram_tensor("XR", [BL, H, S, P], F32, kind="ExternalInput")
    XS = nc.dram_tensor("XS", [BL, H, S, P], F32, kind="ExternalInput")
    A_R = nc.dram_tensor("A_R", [P, P], F32R, kind="ExternalInput")
    A_S = nc.dram_tensor("A_S", [P, P], F32R, kind="ExternalInput")
    W2T = nc.dram_tensor("W2T", [P, P], F32R, kind="ExternalInput")
    C1 = nc.dram_tensor("C1", [P, 1], F32, kind="ExternalInput")
    B2 = nc.dram_tensor("B2", [P, 1], F32, kind="ExternalInput")
    SELW = nc.dram_tensor("SELW", [NC, 8], F32, kind="ExternalInput")
    BDW = nc.dram_tensor("BDW", [1, 8], F32, kind="ExternalInput")
    GATES = nc.dram_tensor("GATES", [1, BL, P], F32, kind="ExternalInput")
    IDENT = nc.dram_tensor("IDENT", [P, P], F32, kind="ExternalInput")
    OUT = nc.dram_tensor("OUT", [BL, H, P, S], F32R, kind="ExternalOutput")

    with tile.TileContext(nc) as tc:
        with (
            tc.tile_pool(name="singles", bufs=1) as singles,
            tc.tile_pool(name="xnat_r", bufs=3) as xnat_r_pool,
            tc.tile_pool(name="xnat_s", bufs=3) as xnat_s_pool,
            tc.tile_pool(name="xrT", bufs=min(H + 2, 10)) as xrT_pool,
            tc.tile_pool(name="fusedT", bufs=min(H + 2, 10)) as fused_pool,
            tc.tile_pool(name="xsT", bufs=3) as xsT_pool,
            tc.tile_pool(name="hsb", bufs=3) as h_pool,
            tc.tile_pool(name="pp", bufs=2) as pp_pool,
            tc.tile_pool(name="small", bufs=4) as small_pool,
            tc.tile_pool(name="ps_r", bufs=2, space="PSUM") as ps_r_pool,
            tc.tile_pool(name="ps_s", bufs=2, space="PSUM") as ps_s_pool,
            tc.tile_pool(name="ps_z", bufs=2, space="PSUM") as ps_z_pool,
            tc.tile_pool(name="ps_f", bufs=1, space="PSUM") as ps_f_pool,
            tc.tile_pool(name="ps_tiny", bufs=1, space="PSUM") as ps_tiny_pool,
        ):
            # --- constants into SBUF (once) ---
            a_r = singles.tile([P, P], F32R)
            nc.sync.dma_start(out=a_r, in_=A_R[:, :])
            a_s = singles.tile([P, P], F32R)
            nc.sync.dma_start(out=a_s, in_=A_S[:, :])
            w2 = singles.tile([P, P], F32R)
            nc.sync.dma_start(out=w2, in_=W2T[:, :])
            c1 = singles.tile([P, 1], F32)
            nc.sync.dma_start(out=c1, in_=C1[:, :])
            b2 = singles.tile([P, 1], F32)
            nc.sync.dma_start(out=b2, in_=B2[:, :])
            selw = singles.tile([NC, 8], F32)
            nc.sync.dma_start(out=selw, in_=SELW[:, :])
            bdw = singles.tile([1, 8], F32)
            nc.sync.dma_start(out=bdw, in_=BDW[:, :])
            gates = singles.tile([1, BL, P], F32)
            nc.sync.dma_start(out=gates, in_=GATES[:, :, :])
            ident = singles.tile([P, P], F32)
            nc.sync.dma_start(out=ident, in_=IDENT[:, :])
            ones = singles.tile([P, 1], F32R)
            nc.vector.memset(ones, 1.0)

            for bl in range(BL):
                xrT_tiles = []
                fusedT_tiles = []
                pool_parts = pp_pool.tile([P, NC], F32R, tag="pp")

                # ---------------- main pass over heads ----------------
                for h in range(H):
                    xr_nat = xnat_r_pool.tile([P, NT, P], F32, tag="xr_nat")
                    nc.sync.dma_start(
                        out=xr_nat,
                        in_=XR[bl, h].rearrange("(p n) d -> p n d", p=P),
                    )
                    xs_nat = xnat_s_pool.tile([P, NT, P], F32, tag="xs_nat")
                    nc.sync.dma_start(
                        out=xs_nat,
                        in_=XS[bl, h].rearrange("(p n) d -> p n d", p=P),
                    )

                    xrT = xrT_pool.tile([P, S], F32R, tag="xrT")
                    fusedT = fused_pool.tile([P, S], BF16, tag="fusedT")
                    xrT_tiles.append(xrT)
                    fusedT_tiles.append(fusedT)

                    for st in range(ST):
                        sl = slice(st * W, (st + 1) * W)
                        psr = ps_r_pool.tile([P, 4, P], F32, tag="ps_r")
                        pss = ps_s_pool.tile([P, 4, P], F32, tag="ps_s")
                        for k in range(4):
                            n0 = st * 4 + k
                            nc.tensor.transpose(psr[:, k, :], xr_nat[:, n0, :], ident)
                            nc.tensor.transpose(pss[:, k, :], xs_nat[:, n0, :], ident)
                        nc.scalar.copy(
                            out=xrT[:, sl].rearrange("d (k p) -> d k p", k=4),
                            in_=psr,
                        )
                        xsT = xsT_pool.tile([P, 4, P], F32R, tag="xsT")
                        nc.scalar.copy(out=xsT, in_=pss)

                        pz = ps_z_pool.tile([P, W], F32, tag="ps_z")
                        nc.tensor.matmul(
                            pz, a_r, xrT[:, sl],
                            start=True, stop=False,
                        )
                        nc.tensor.matmul(
                            pz, a_s,
                            xsT.rearrange("d k p -> d (k p)"),
                            start=False, stop=True,
                        )
                        hsb = h_pool.tile([P, W], F32R, tag="hsb")
                        nc.scalar.activation(
                            out=hsb, in_=pz, func=act_func, bias=c1, scale=1.0,
                        )
                        pf = ps_f_pool.tile([P, W], F32, tag="ps_f")
                        nc.tensor.matmul(
                            pf, w2, hsb,
                            start=True, stop=True,
                        )
                        col = h * ST + st
                        nc.vector.tensor_scalar(
                            out=fusedT[:, sl], in0=pf,
                            scalar1=b2, scalar2=None, op0=OP.add, op1=OP.add,
                            accum_out=pool_parts[:, col:col + 1],
                        )

                # ---------------- squeeze-excite scale ----------------
                pt1 = ps_tiny_pool.tile([NC, 1], F32, tag="tiny")
                nc.tensor.matmul(
                    pt1, pool_parts, ones,
                    start=True, stop=True,
                )
                colsum = small_pool.tile([NC, 1], F32R, tag="colsum")
                nc.vector.tensor_copy(colsum, pt1)
                pt2 = ps_tiny_pool.tile([1, 8], F32, tag="tiny")
                nc.tensor.matmul(
                    pt2, colsum, selw,
                    start=True, stop=True,
                )
                pre = small_pool.tile([1, 8], F32, tag="pre")
                nc.vector.tensor_add(pre, pt2, bdw)
                scale = small_pool.tile([1, 8], F32, tag="scale")
                nc.scalar.activation(out=scale, in_=pre, func=AF.Sigmoid)
                pt3 = ps_tiny_pool.tile([P, 8], F32, tag="tiny")
                nc.tensor.matmul(
                    pt3, gates[0:1, bl, :], scale, start=True, stop=True,
                )
                c_sb = small_pool.tile([P, 8], F32, tag="c_sb")
                nc.vector.tensor_copy(c_sb, pt3)

                # ---------------- epilogue: residual + store ----------------
                for h in range(H):
                    xrT = xrT_tiles[h]
                    fusedT = fusedT_tiles[h]
                    for st in range(ST):
                        sl = slice(st * W, (st + 1) * W)
                        nc.vector.scalar_tensor_tensor(
                            out=xrT[:, sl], in0=fusedT[:, sl],
                            scalar=c_sb[:, h:h + 1], in1=xrT[:, sl],
                            op0=OP.mult, op1=OP.add,
                        )
                    nc.gpsimd.dma_start(out=OUT[bl, h], in_=xrT)

    return nc


# ---------------------------------------------------------------------------
# host side
# ---------------------------------------------------------------------------

_NC_CACHE: dict = {}


def _get_module(BL, H, S):
    key = (BL, H, S)
    if key not in _NC_CACHE:
        nc = build_module(BL, H, S)
        nc.compile()
        _NC_CACHE[key] = nc
    return _NC_CACHE[key]


def _round_fp22(a):
    """Round-to-nearest-even to 13 mantissa bits (fp32r's fp22 read precision)."""
    u = np.asarray(a, np.float32).view(np.uint32).astype(np.uint64)
    u = (u + np.uint64(0x1000) + ((u >> np.uint64(13)) & np.uint64(1))) >> np.uint64(13)
    return (u.astype(np.uint32) << np.uint32(13)).view(np.float32)


def _host_constants(Wp, bp, W1, b1, W2, b2, Wdw, bdw, gate_param, BL, H, S, ST):
    """Per-branch constant tensors (fp32), using float64 intermediates."""
    A = (Wp.astype(np.float64) @ W1.astype(np.float64)).astype(np.float32)
    c1 = (bp.astype(np.float64) @ W1.astype(np.float64)).astype(np.float32) + b1
    selw = np.repeat(Wdw.T.astype(np.float32), ST, axis=0) / np.float32(S * P)
    A = _round_fp22(A)
    selw = _round_fp22(selw)
    g = -np.log(-np.log(_GUMBEL_U[: gate_param.shape[0]] + _EPS) + _EPS)
    gate = 1.0 / (1.0 + np.exp(-((gate_param + g) / _TEMPERATURE)))
    gate = gate.astype(np.float32)
    return {
        "A_R": np.ascontiguousarray(A[:P]),
        "A_S": np.ascontiguousarray(A[P:]),
        "W2T": np.ascontiguousarray(W2.astype(np.float32)),
        "C1": c1.reshape(P, 1).astype(np.float32),
        "B2": b2.reshape(P, 1).astype(np.float32),
        "SELW": np.ascontiguousarray(selw),
        "BDW": bdw.reshape(1, 8).astype(np.float32),
        "IDENT": np.eye(P, dtype=np.float32),
        "gate": gate,
    }


def kernel(K_R, V_R, K_S, V_S, Wk, bk, Wv, bv, W1, b1, W2, b2, Wdw, bdw,
           gate_param, _trace=False):
    K_R, V_R, K_S, V_S = (np.asarray(t, dtype=np.float32) for t in (K_R, V_R, K_S, V_S))
    Wk, bk, Wv, bv, W1, b1, W2, b2, Wdw, bdw, gate_param = (
        np.asarray(t, dtype=np.float32)
        for t in (Wk, bk, Wv, bv, W1, b1, W2, b2, Wdw, bdw, gate_param)
    )
    L, B, H, S, DR = K_R.shape
    assert B == 1 and DR == P and H * S % W == 0
    n_cores = 8
    BL = 2 * L // n_cores          # branch-layers per core
    ST = S // W

    nc = _get_module(BL, H, S)

    ck = _host_constants(Wk, bk, W1, b1, W2, b2, Wdw, bdw, gate_param,
                         BL, H, S, ST)
    cv = _host_constants(Wv, bv, W1, b1, W2, b2, Wdw, bdw, gate_param,
                         BL, H, S, ST)
    gate = ck["gate"]  # identical in both

    in_maps = []
    for c in range(n_cores):
        branch_k = c < n_cores // 2
        lc = c if branch_k else c - n_cores // 2
        lsl = slice(lc * BL, (lc + 1) * BL)
        cc = ck if branch_k else cv
        gates_np = np.broadcast_to(
            gate[lsl].reshape(1, BL, 1), (1, BL, P)
        ).astype(np.float32).copy()
        in_maps.append({
            "XR": (K_R if branch_k else V_R)[lsl, 0],
            "XS": (K_S if branch_k else V_S)[lsl, 0],
            "A_R": cc["A_R"], "A_S": cc["A_S"], "W2T": cc["W2T"],
            "C1": cc["C1"], "B2": cc["B2"], "SELW": cc["SELW"],
            "BDW": cc["BDW"], "GATES": gates_np, "IDENT": cc["IDENT"],
        })

    res = run_bass_kernel_spmd(nc, in_maps, core_ids=list(range(n_cores)),
                               trace=_trace)
    kernel.last_results = res

    out = np.empty((2, L, 1, H, S, P), dtype=np.float32)
    NT = S // P
    for c in range(n_cores):
        branch_k = c < n_cores // 2
        lc = c if branch_k else c - n_cores // 2
        r = res.results[c]["OUT"]          # [BL, H, P(d), S(j)] ; j = n0*128 + p
        r = r.reshape(BL, H, P, NT, P)     # (bl, h, d, n0, p)
        r = r.transpose(0, 1, 4, 3, 2).reshape(BL, H, S, P)  # s = p*NT + n0
        out[0 if branch_k else 1, lc * BL:(lc + 1) * BL, 0] = r
    return out
